# revision 35
# baseline (speedup 1.0000x reference)
# Trainium2 Bass kernel for nn_DetectionLoss (B=32, N=25200, M=200, C=80).
#
# Strategy: pure data-parallel over batch (4 batches per core, 8 cores).
# The reference only reads pred_bbox[:, :M] and pred_cls[:, :M], so only
# those slices are shipped to the device. Each core computes per-partition
# partial sums of the loss terms; the host does the final (tiny) cross-core
# reduction and mean/lambda arithmetic in float64.
#
# Device inputs per core (host-packed; fp8e3 keeps ~1% per-element error
# that averages out over the 6400..100800-element means; boxes stay f32
# because the near-zero enclose/union denominators amplify input rounding):
#   obj  [128, 800] fp8e3: rows 0:126 all 4*25200 obj logits (flat),
#                          row 126 = -pos logits, row 127 = +pos logits
#   cls  [100, 640] fp8e3: cls logits [p, a=8, c=80]
#   small[100, 72] f32:    cols 0:64 pred|gt boxes (cx,cy,w,h) [p, s, j, c],
#                          cols 64:72 host-gathered picked logits
# All tensors span >=100 partitions so every transfer engages most of the
# 16 SDMA engines; obj is split into two 64-row DMAs across the queues.
#
# On-device: softplus as exp -> ln(x*1+1) (the +1 rides the Ln activation's
# pre-bias, no elementwise pass); lse as exp -> DVE reduce -> ln with accum;
# GIoU on DVE; per-partition partials collapse to [3, 8] via a PE matmul
# with an iota-built selector so the output DMA is 3 descriptors:
#   row 0 = sums over partitions 0:126, row 1 = partition 126 (softplus(-pos)
#   sum), row 2 = partition 127 (softplus(+pos) sum); cols as listed above:
#   0 iou, 1 (enclose-union)/(enclose+eps), 2 softplus, 3 lse, 4 picked.

import numpy as np

B, N, M, C = 32, 25200, 200, 80
NCORES = 8
BPC = B // NCORES          # 4 batches per core
KP = 2                     # anchors per (partition, batch) for the box tile
P_PAIRS = M // KP          # 100 partitions for pair-space tiles
NPAIR = BPC * KP           # 8 pairs per partition
P_OBJ, F_OBJ = 126, 800    # 4*25200 = 126*800 exactly
NANCH = BPC * M // P_PAIRS # 8 anchors per cls partition
W_CLS = NANCH * C          # 640
EPS = 1e-7

_CACHED_NC = None


def _emit(nc, tc, mybir, obj, cls_t, small, out):
    f32 = mybir.dt.float32
    Alu = mybir.AluOpType
    Act = mybir.ActivationFunctionType

    with tc.tile_pool(name="main", bufs=1) as pool:
        ACC = pool.tile([128, 8], f32, name="ACC")
        nc.vector.memset(ACC[:], 0.0)
        # Anchor: a dependency-free 1-col Exp so insert_act_table_loads places
        # the (1.3us) ACT_TABLE_LOAD here, overlapping the input DMAs, instead
        # of sandwiched between the first real activation's DMA waits.
        DUM = pool.tile([128, 1], f32, name="DUM")
        nc.scalar.activation(DUM[:], ACC[:, 7:8], Act.Exp)

        OBJ = pool.tile([128, F_OBJ], mybir.dt.float8e3, name="OBJ")
        CLS = pool.tile([P_PAIRS, W_CLS], mybir.dt.float8e3, name="CLS")
        SM = pool.tile([P_PAIRS, 72], f32, name="SM")
        # First slots: small on sync (starts the DVE GIoU chain earliest),
        # cls on scalar (the scheduler runs exp_cls first); obj halves take
        # the two second slots so the full [128,800] tile lands ~10.1us.
        nc.sync.dma_start(out=SM[:], in_=small.ap())
        nc.scalar.dma_start(out=CLS[:], in_=cls_t.ap())
        nc.sync.dma_start(out=OBJ[0:64], in_=obj.ap()[0:64])
        nc.scalar.dma_start(out=OBJ[64:128], in_=obj.ap()[64:128])

        # ---------------- objectness softplus ----------------
        # sum softplus(x): exp on ACT, then Ln with pre-activation bias=1.0
        # (out = ln(in*1 + 1)) with accum_out — no elementwise +1 pass needed.
        Eo = pool.tile([128, F_OBJ], f32, name="Eo")
        Lg = pool.tile([128, F_OBJ], f32, name="Lg")
        nc.scalar.activation(Eo[:], OBJ[:], Act.Exp)

        # ---------------- classification ----------------
        Ec = pool.tile([P_PAIRS, NANCH, C], mybir.dt.bfloat16, name="Ec")
        sums = pool.tile([P_PAIRS, NANCH], f32, name="sums")
        lse = pool.tile([P_PAIRS, NANCH], f32, name="lse")
        nc.scalar.activation(
            Ec[:].rearrange("p a c -> p (a c)"), CLS[:], Act.Exp
        )
        nc.scalar.activation(Lg[:], Eo[:], Act.Ln, bias=1.0,
                             accum_out=ACC[0:128, 2:3])

        # ---------------- bbox GIoU term ----------------
        # boxes ship as f32 (cx,cy,w,h): the near-zero enclose/union
        # denominators amplify any input rounding, so this path must match
        # the reference's f32 arithmetic exactly.
        PB = SM[:, 0:64].rearrange("p (s j c) -> p s j c", s=2, c=4)
        cxcy = PB[:, :, :, 0:2]
        wh = PB[:, :, :, 2:4]
        C1 = pool.tile([P_PAIRS, 2, NPAIR, 2], f32, name="C1")
        C2 = pool.tile([P_PAIRS, 2, NPAIR, 2], f32, name="C2")
        nc.vector.scalar_tensor_tensor(C1[:], wh, -0.5, cxcy, Alu.mult, Alu.add)
        nc.vector.scalar_tensor_tensor(C2[:], wh, 0.5, cxcy, Alu.mult, Alu.add)
        I1 = pool.tile([P_PAIRS, NPAIR, 2], f32, name="I1")
        I2 = pool.tile([P_PAIRS, NPAIR, 2], f32, name="I2")
        E1 = pool.tile([P_PAIRS, NPAIR, 2], f32, name="E1")
        E2 = pool.tile([P_PAIRS, NPAIR, 2], f32, name="E2")
        nc.vector.tensor_tensor(I1[:], C1[:, 0], C1[:, 1], Alu.max)
        nc.vector.tensor_tensor(I2[:], C2[:, 0], C2[:, 1], Alu.min)
        nc.vector.tensor_tensor(E1[:], C1[:, 0], C1[:, 1], Alu.min)
        nc.vector.tensor_tensor(E2[:], C2[:, 0], C2[:, 1], Alu.max)
        ID = pool.tile([P_PAIRS, NPAIR, 2], f32, name="ID")
        IDr = pool.tile([P_PAIRS, NPAIR, 2], f32, name="IDr")
        ED = pool.tile([P_PAIRS, NPAIR, 2], f32, name="ED")
        nc.vector.tensor_sub(ID[:], I2[:], I1[:])
        nc.vector.tensor_relu(IDr[:], ID[:])
        nc.vector.tensor_sub(ED[:], E2[:], E1[:])
        inter = pool.tile([P_PAIRS, NPAIR], f32, name="inter")
        encl = pool.tile([P_PAIRS, NPAIR], f32, name="encl")
        nc.vector.tensor_mul(inter[:], IDr[:, :, 0], IDr[:, :, 1])
        nc.vector.tensor_mul(encl[:], ED[:, :, 0], ED[:, :, 1])
        A = pool.tile([P_PAIRS, 2, NPAIR], f32, name="A")
        nc.vector.tensor_mul(A[:], PB[:, :, :, 2], PB[:, :, :, 3])
        asum = pool.tile([P_PAIRS, NPAIR], f32, name="asum")
        nc.vector.tensor_add(asum[:], A[:, 0], A[:, 1])
        U = pool.tile([P_PAIRS, NPAIR], f32, name="U")
        nc.vector.scalar_tensor_tensor(U[:], inter[:], -1.0, asum[:],
                                       Alu.mult, Alu.add)
        # U+eps and encl+eps laid adjacently so one reciprocal covers both
        R = pool.tile([P_PAIRS, 2, NPAIR], f32, name="R")
        R2 = pool.tile([P_PAIRS, 2, NPAIR], f32, name="R2")
        nc.vector.tensor_scalar_add(R[:, 0], U[:], EPS)
        nc.vector.tensor_scalar_add(R[:, 1], encl[:], EPS)
        nc.vector.reciprocal(R2[:], R[:])
        # NOTE: tensor_tensor_reduce wedges the device (NRT_EXEC_UNIT_UNRECOVERABLE)
        # on this runtime; scalar_tensor_tensor's accum_out path works.
        t8a = pool.tile([P_PAIRS, NPAIR], f32, name="t8a")
        nc.vector.scalar_tensor_tensor(
            t8a[:], inter[:], 1.0, R2[:, 0], Alu.mult, Alu.mult,
            accum_out=ACC[0:P_PAIRS, 0:1],
        )
        EmU = pool.tile([P_PAIRS, NPAIR], f32, name="EmU")
        nc.vector.tensor_sub(EmU[:], encl[:], U[:])
        t8b = pool.tile([P_PAIRS, NPAIR], f32, name="t8b")
        nc.vector.scalar_tensor_tensor(
            t8b[:], EmU[:], 1.0, R2[:, 1], Alu.mult, Alu.mult,
            accum_out=ACC[0:P_PAIRS, 1:2],
        )

        # cls tail: DVE reduce after the GIoU chain, then Ln on ACT
        nc.vector.reduce_sum(out=sums[:], in_=Ec[:], axis=mybir.AxisListType.X)
        nc.scalar.activation(lse[:], sums[:], Act.Ln,
                             accum_out=ACC[0:P_PAIRS, 3:4])
        # picked-logit sum: plain DVE reduce straight into the ACC column
        nc.vector.reduce_sum(out=ACC[0:P_PAIRS, 4:5], in_=SM[:, 64:72],
                             axis=mybir.AxisListType.X)

        # Collapse ACC [128,8] to [3,8] on the idle PE so the output DMA is
        # 3 descriptors instead of 128 (cuts its HBM receipt latency, which
        # sits on the critical path into the NEFF epilogue barrier):
        # row 0 = sum over partitions 0:126, row 1 = partition 126 (-pos
        # softplus), row 2 = partition 127 (+pos softplus).
        W = pool.tile([128, 3], f32, name="W")
        IOT = pool.tile([128, 1], mybir.dt.int32, name="IOT")
        nc.gpsimd.iota(IOT[:], pattern=[[0, 1]], base=0, channel_multiplier=1)
        nc.vector.tensor_scalar(W[:, 0:1], IOT[:], P_OBJ, None,
                                op0=Alu.is_lt)
        nc.vector.tensor_scalar(W[:, 1:2], IOT[:], P_OBJ, None,
                                op0=Alu.is_equal)
        nc.vector.tensor_scalar(W[:, 2:3], IOT[:], P_OBJ + 1, None,
                                op0=Alu.is_equal)
        from concourse.bass import MemorySpace
        PS, _ps_free = tc.tile([3, 8], f32, space=MemorySpace.PSUM, name="PS")
        nc.tensor.matmul(PS[:], lhsT=W[:], rhs=ACC[:], start=True, stop=True)
        FIN = pool.tile([3, 8], f32, name="FIN")
        nc.vector.tensor_copy(FIN[:], PS[:])

        nc.sync.dma_start(out=out.ap(), in_=FIN[:])



def build_bass():
    global _CACHED_NC
    if _CACHED_NC is not None:
        return _CACHED_NC
    import concourse.bacc as bacc
    import concourse.tile as tile
    import concourse.mybir as mybir
    import concourse.bass_utils as _bu

    # The NEFF epilogue zeroes the whole sem file one EVENT_SEMAPHORE at a
    # time (~6us, serialized on the slowest engine). --max-sem-num=150 trims
    # the first few walrus-internal sems out of that clear set (3..6).
    if not hasattr(_bu, "_orig_get_walrus_args"):
        _bu._orig_get_walrus_args = _bu.get_walrus_args

        def _patched_walrus_args(*a, **k):
            return [*_bu._orig_get_walrus_args(*a, **k), "--max-sem-num=150"]

        _bu.get_walrus_args = _patched_walrus_args

    f32 = mybir.dt.float32
    bf16 = mybir.dt.bfloat16
    fp8 = mybir.dt.float8e3
    Act = mybir.ActivationFunctionType

    class FastTileContext(tile.TileContext):
        # TileContext._drain_and_barrier minus the end-of-kernel barrier and
        # the semaphore range-clear — the NEFF's own epilogue already runs a
        # pre-storm all-engine barrier, and with every kernel sem steered
        # into 207..255 the storm's Sync-engine portion (which runs after
        # Sync's drain, i.e. after the output DMA lands) covers the reset.
        def _drain_and_barrier(self, tick_clock, wait_clock):
            clock = tile.ScopedClock({None: tick_clock.global_clock})
            drain_inst = self.nc.sync.drain()
            wait_clock.add_sem_waits(drain_inst.ins, clock)
            popped = self.nc._tile_sem_poison_stack.pop()
            assert popped is self._sem_poison
            # No barrier and no explicit sem clear: the NEFF's own epilogue
            # runs a pre-storm all-engine barrier, and its Sync-engine clear
            # range (207..255) covers every sem this kernel uses.

    nc = bacc.Bacc("TRN2", target_bir_lowering=False, debug=False,
                   num_devices=NCORES)
    # Steer every tile/DMA semaphore into 207..255: keeps the live sems out
    # of the clear ranges the NEFF epilogue hands to the non-Sync engines.
    nc._state.reset_free_semaphores(list(range(207, 256)))
    obj = nc.dram_tensor("obj", [128, F_OBJ], fp8, kind="ExternalInput")
    cls_t = nc.dram_tensor("cls", [P_PAIRS, W_CLS], fp8, kind="ExternalInput")
    small = nc.dram_tensor("small", [P_PAIRS, 72], f32, kind="ExternalInput")
    out = nc.dram_tensor("partials", [3, 8], f32, kind="ExternalOutput")
    with FastTileContext(nc) as tc:
        _emit(nc, tc, mybir, obj, cls_t, small, out)

    # Route every Exp/Ln to the one table that holds both, so the kernel pays
    # a single ACT_TABLE_LOAD instead of ping-ponging between per-func tables.
    orig_tables = bacc.get_activation_tables

    def _merged_tables(arch):
        out_d = {}
        for name, s in orig_tables(arch).items():
            s2 = set(s)
            if name != "natural_log_exp_and_others":
                s2.discard(Act.Exp)
                s2.discard(Act.Ln)
            out_d[name] = s2
        return out_d

    bacc.get_activation_tables = _merged_tables
    try:
        nc.compile()
    finally:
        bacc.get_activation_tables = orig_tables

    # Drop the two dead const memsets (bf16 1.0, uint8 127 — this kernel
    # never reads them): the gpsimd const chain gates the tile-enter dance,
    # so each dead memset costs ~0.1us of every core's prologue.
    entry = nc.main_func.blocks[0]
    dead_consts = ("const-bfloat16-1.0", "const-uint8-127")
    entry.instructions[:] = [
        ins for ins in entry.instructions
        if not (type(ins).__name__ == "InstMemset"
                and getattr(ins, "outs", None)
                and any(d in str(ins.outs[0]) for d in dead_consts))
    ]

    # Drop a spurious default-table InstLoadActFuncSet: when two loads appear
    # with no activation between them, the first is dead and its 1.3us sits
    # right before the first Exp on the critical path.
    for blk in nc.main_func.blocks:
        loads = []
        acts_seen = set()
        for idx, ins in enumerate(blk.instructions):
            tn = type(ins).__name__
            if tn == "InstLoadActFuncSet":
                loads.append((idx, ins))
            elif tn == "InstActivation":
                acts_seen.add(len(loads))
        if len(loads) == 2 and 1 not in acts_seen and loads[0][1].sync_info is None:
            blk.instructions.pop(loads[0][0])

    _CACHED_NC = nc
    return nc


def make_in_maps(pred_bbox, pred_obj, pred_cls, gt_boxes, gt_labels):
    import ml_dtypes

    fp8 = ml_dtypes.float8_e3m4
    labels = np.asarray(gt_labels).astype(np.int64)
    in_maps = []
    for core in range(NCORES):
        bs = slice(core * BPC, (core + 1) * BPC)

        po = np.asarray(pred_obj[bs], np.float32)
        obj = np.empty((128, F_OBJ), np.float32)
        obj[0:P_OBJ] = po.reshape(P_OBJ, F_OBJ)
        obj[P_OBJ] = -po[:, :M].reshape(F_OBJ)
        obj[P_OBJ + 1] = po[:, :M].reshape(F_OBJ)

        cl = np.asarray(pred_cls[bs, :M], np.float32).reshape(P_PAIRS, W_CLS)

        small = np.empty((P_PAIRS, 72), np.float32)
        pb = np.asarray(pred_bbox[bs, :M], np.float32).reshape(BPC, P_PAIRS, KP, 4)
        gb = np.asarray(gt_boxes[bs], np.float32).reshape(BPC, P_PAIRS, KP, 4)
        small[:, 0:32] = pb.transpose(1, 0, 2, 3).reshape(P_PAIRS, 32)
        small[:, 32:64] = gb.transpose(1, 0, 2, 3).reshape(P_PAIRS, 32)
        picked = np.take_along_axis(
            np.asarray(pred_cls[bs, :M], np.float32),
            labels[bs][..., None].astype(np.int64), axis=-1,
        )[..., 0]
        small[:, 64:72] = picked.reshape(P_PAIRS, NANCH)

        in_maps.append({
            "obj": obj.astype(fp8),
            "cls": cl.astype(fp8),
            "small": small,
        })
    return in_maps


def finalize(per_core_partials):
    s_iou = s_ratio = s_all = s_pos = s_posplus = s_lse = s_picked = 0.0
    for p in per_core_partials:
        p = p.astype(np.float64)
        s_iou += p[0, 0]
        s_ratio += p[0, 1]
        s_all += p[0, 2]
        s_pos += p[1, 2]
        s_posplus += p[2, 2]
        s_lse += p[0, 3]
        s_picked += p[0, 4]
    n_pos = B * M
    n_neg = B * (N - M)
    loss_bbox = 5.0 * (n_pos - s_iou + s_ratio) / n_pos
    loss_obj = s_pos / n_pos + 0.5 * (s_all - s_posplus) / n_neg
    loss_cls = (s_lse - s_picked) / n_pos
    total = loss_bbox + loss_obj + loss_cls
    return np.array([total, loss_bbox, loss_obj, loss_cls], dtype=np.float32)


def kernel(pred_bbox, pred_obj, pred_cls, gt_boxes, gt_labels):
    from concourse.bass_utils import run_bass_kernel_spmd

    nc = build_bass()
    in_maps = make_in_maps(pred_bbox, pred_obj, pred_cls, gt_boxes, gt_labels)
    res = run_bass_kernel_spmd(nc, in_maps, core_ids=list(range(NCORES)))
    return finalize([r["partials"] for r in res.results])


# revision 37
# speedup vs baseline: 1.0032x; 1.0032x over previous
# Trainium2 Bass kernel for nn_DetectionLoss (B=32, N=25200, M=200, C=80).
#
# Strategy: pure data-parallel over batch (4 batches per core, 8 cores).
# The reference only reads pred_bbox[:, :M] and pred_cls[:, :M], so only
# those slices are shipped to the device. Each core computes per-partition
# partial sums of the loss terms; the host does the final (tiny) cross-core
# reduction and mean/lambda arithmetic in float64.
#
# Device inputs per core (host-packed; fp8e3 keeps ~1% per-element error
# that averages out over the 6400..100800-element means; boxes stay f32
# because the near-zero enclose/union denominators amplify input rounding):
#   obj  [128, 800] fp8e3: rows 0:126 all 4*25200 obj logits (flat),
#                          row 126 = -pos logits, row 127 = +pos logits
#   cls  [100, 640] fp8e3: cls logits [p, a=8, c=80]
#   small[100, 72] f32:    cols 0:64 pred|gt boxes (cx,cy,w,h) [p, s, j, c],
#                          cols 64:72 host-gathered picked logits
# All tensors span >=100 partitions so every transfer engages most of the
# 16 SDMA engines; obj is split into two 64-row DMAs across the queues.
#
# On-device: softplus as exp -> ln(x*1+1) (the +1 rides the Ln activation's
# pre-bias, no elementwise pass); lse as exp -> DVE reduce -> ln with accum;
# GIoU on DVE; per-partition partials collapse to [3, 8] via a PE matmul
# with an iota-built selector so the output DMA is 3 descriptors:
#   row 0 = sums over partitions 0:126, row 1 = partition 126 (softplus(-pos)
#   sum), row 2 = partition 127 (softplus(+pos) sum); cols as listed above:
#   0 iou, 1 (enclose-union)/(enclose+eps), 2 softplus, 3 lse, 4 picked.

import numpy as np

B, N, M, C = 32, 25200, 200, 80
NCORES = 8
BPC = B // NCORES          # 4 batches per core
KP = 2                     # anchors per (partition, batch) for the box tile
P_PAIRS = M // KP          # 100 partitions for pair-space tiles
NPAIR = BPC * KP           # 8 pairs per partition
P_OBJ, F_OBJ = 126, 800    # 4*25200 = 126*800 exactly
NANCH = BPC * M // P_PAIRS # 8 anchors per cls partition
W_CLS = NANCH * C          # 640
EPS = 1e-7

_CACHED_NC = None


def _emit(nc, tc, mybir, obj, cls_t, small, out):
    f32 = mybir.dt.float32
    Alu = mybir.AluOpType
    Act = mybir.ActivationFunctionType

    with tc.tile_pool(name="main", bufs=1) as pool:
        ACC = pool.tile([128, 8], f32, name="ACC")
        nc.vector.memset(ACC[:], 0.0)
        # Activation bias constants built in-block on DVE: the Bass preamble's
        # gpsimd const memsets gate the tile-enter dance, so registering our
        # own lets the (now-dead) preamble ones be stripped after compile.
        CB = pool.tile([128, 2], f32, name="CB")
        nc.vector.memset(CB[:, 0:1], 0.0)
        nc.vector.memset(CB[:, 1:2], 1.0)
        # Anchor: a dependency-free 1-col Exp so insert_act_table_loads places
        # the (1.3us) ACT_TABLE_LOAD here, overlapping the input DMAs, instead
        # of sandwiched between the first real activation's DMA waits.
        DUM = pool.tile([128, 1], f32, name="DUM")
        nc.scalar.activation(DUM[:], ACC[:, 7:8], Act.Exp, bias=CB[:, 0:1])

        OBJ = pool.tile([128, F_OBJ], mybir.dt.float8e3, name="OBJ")
        CLS = pool.tile([P_PAIRS, W_CLS], mybir.dt.float8e3, name="CLS")
        SM = pool.tile([P_PAIRS, 72], f32, name="SM")
        # First slots: small on sync (starts the DVE GIoU chain earliest),
        # cls on scalar (the scheduler runs exp_cls first); obj halves take
        # the two second slots so the full [128,800] tile lands ~10.1us.
        nc.sync.dma_start(out=SM[:], in_=small.ap())
        nc.scalar.dma_start(out=CLS[:], in_=cls_t.ap())
        nc.sync.dma_start(out=OBJ[0:64], in_=obj.ap()[0:64])
        nc.scalar.dma_start(out=OBJ[64:128], in_=obj.ap()[64:128])

        # ---------------- objectness softplus ----------------
        # sum softplus(x): exp on ACT, then Ln with pre-activation bias=1.0
        # (out = ln(in*1 + 1)) with accum_out — no elementwise +1 pass needed.
        Eo = pool.tile([128, F_OBJ], f32, name="Eo")
        Lg = pool.tile([128, F_OBJ], f32, name="Lg")
        nc.scalar.activation(Eo[:], OBJ[:], Act.Exp, bias=CB[:, 0:1])

        # ---------------- classification ----------------
        Ec = pool.tile([P_PAIRS, NANCH, C], mybir.dt.bfloat16, name="Ec")
        sums = pool.tile([P_PAIRS, NANCH], f32, name="sums")
        lse = pool.tile([P_PAIRS, NANCH], f32, name="lse")
        nc.scalar.activation(
            Ec[:].rearrange("p a c -> p (a c)"), CLS[:], Act.Exp,
            bias=CB[0:P_PAIRS, 0:1],
        )
        nc.scalar.activation(Lg[:], Eo[:], Act.Ln, bias=CB[:, 1:2],
                             accum_out=ACC[0:128, 2:3])

        # ---------------- bbox GIoU term ----------------
        # boxes ship as f32 (cx,cy,w,h): the near-zero enclose/union
        # denominators amplify any input rounding, so this path must match
        # the reference's f32 arithmetic exactly.
        PB = SM[:, 0:64].rearrange("p (s j c) -> p s j c", s=2, c=4)
        cxcy = PB[:, :, :, 0:2]
        wh = PB[:, :, :, 2:4]
        C1 = pool.tile([P_PAIRS, 2, NPAIR, 2], f32, name="C1")
        C2 = pool.tile([P_PAIRS, 2, NPAIR, 2], f32, name="C2")
        nc.vector.scalar_tensor_tensor(C1[:], wh, -0.5, cxcy, Alu.mult, Alu.add)
        nc.vector.scalar_tensor_tensor(C2[:], wh, 0.5, cxcy, Alu.mult, Alu.add)
        I1 = pool.tile([P_PAIRS, NPAIR, 2], f32, name="I1")
        I2 = pool.tile([P_PAIRS, NPAIR, 2], f32, name="I2")
        E1 = pool.tile([P_PAIRS, NPAIR, 2], f32, name="E1")
        E2 = pool.tile([P_PAIRS, NPAIR, 2], f32, name="E2")
        nc.vector.tensor_tensor(I1[:], C1[:, 0], C1[:, 1], Alu.max)
        nc.vector.tensor_tensor(I2[:], C2[:, 0], C2[:, 1], Alu.min)
        nc.vector.tensor_tensor(E1[:], C1[:, 0], C1[:, 1], Alu.min)
        nc.vector.tensor_tensor(E2[:], C2[:, 0], C2[:, 1], Alu.max)
        ID = pool.tile([P_PAIRS, NPAIR, 2], f32, name="ID")
        IDr = pool.tile([P_PAIRS, NPAIR, 2], f32, name="IDr")
        ED = pool.tile([P_PAIRS, NPAIR, 2], f32, name="ED")
        nc.vector.tensor_sub(ID[:], I2[:], I1[:])
        nc.vector.tensor_relu(IDr[:], ID[:])
        nc.vector.tensor_sub(ED[:], E2[:], E1[:])
        inter = pool.tile([P_PAIRS, NPAIR], f32, name="inter")
        encl = pool.tile([P_PAIRS, NPAIR], f32, name="encl")
        nc.vector.tensor_mul(inter[:], IDr[:, :, 0], IDr[:, :, 1])
        nc.vector.tensor_mul(encl[:], ED[:, :, 0], ED[:, :, 1])
        A = pool.tile([P_PAIRS, 2, NPAIR], f32, name="A")
        nc.vector.tensor_mul(A[:], PB[:, :, :, 2], PB[:, :, :, 3])
        asum = pool.tile([P_PAIRS, NPAIR], f32, name="asum")
        nc.vector.tensor_add(asum[:], A[:, 0], A[:, 1])
        U = pool.tile([P_PAIRS, NPAIR], f32, name="U")
        nc.vector.scalar_tensor_tensor(U[:], inter[:], -1.0, asum[:],
                                       Alu.mult, Alu.add)
        # U+eps and encl+eps laid adjacently so one reciprocal covers both
        R = pool.tile([P_PAIRS, 2, NPAIR], f32, name="R")
        R2 = pool.tile([P_PAIRS, 2, NPAIR], f32, name="R2")
        nc.vector.tensor_scalar_add(R[:, 0], U[:], EPS)
        nc.vector.tensor_scalar_add(R[:, 1], encl[:], EPS)
        nc.vector.reciprocal(R2[:], R[:])
        # NOTE: tensor_tensor_reduce wedges the device (NRT_EXEC_UNIT_UNRECOVERABLE)
        # on this runtime; scalar_tensor_tensor's accum_out path works.
        t8a = pool.tile([P_PAIRS, NPAIR], f32, name="t8a")
        nc.vector.scalar_tensor_tensor(
            t8a[:], inter[:], 1.0, R2[:, 0], Alu.mult, Alu.mult,
            accum_out=ACC[0:P_PAIRS, 0:1],
        )
        EmU = pool.tile([P_PAIRS, NPAIR], f32, name="EmU")
        nc.vector.tensor_sub(EmU[:], encl[:], U[:])
        t8b = pool.tile([P_PAIRS, NPAIR], f32, name="t8b")
        nc.vector.scalar_tensor_tensor(
            t8b[:], EmU[:], 1.0, R2[:, 1], Alu.mult, Alu.mult,
            accum_out=ACC[0:P_PAIRS, 1:2],
        )

        # cls tail: DVE reduce after the GIoU chain, then Ln on ACT
        nc.vector.reduce_sum(out=sums[:], in_=Ec[:], axis=mybir.AxisListType.X)
        nc.scalar.activation(lse[:], sums[:], Act.Ln,
                             bias=CB[0:P_PAIRS, 0:1],
                             accum_out=ACC[0:P_PAIRS, 3:4])
        # picked-logit sum: plain DVE reduce straight into the ACC column
        nc.vector.reduce_sum(out=ACC[0:P_PAIRS, 4:5], in_=SM[:, 64:72],
                             axis=mybir.AxisListType.X)

        # Collapse ACC [128,8] to [3,8] on the idle PE so the output DMA is
        # 3 descriptors instead of 128 (cuts its HBM receipt latency, which
        # sits on the critical path into the NEFF epilogue barrier):
        # row 0 = sum over partitions 0:126, row 1 = partition 126 (-pos
        # softplus), row 2 = partition 127 (+pos softplus).
        W = pool.tile([128, 3], f32, name="W")
        IOT = pool.tile([128, 1], mybir.dt.int32, name="IOT")
        nc.gpsimd.iota(IOT[:], pattern=[[0, 1]], base=0, channel_multiplier=1)
        nc.vector.tensor_scalar(W[:, 0:1], IOT[:], P_OBJ, None,
                                op0=Alu.is_lt)
        nc.vector.tensor_scalar(W[:, 1:2], IOT[:], P_OBJ, None,
                                op0=Alu.is_equal)
        nc.vector.tensor_scalar(W[:, 2:3], IOT[:], P_OBJ + 1, None,
                                op0=Alu.is_equal)
        from concourse.bass import MemorySpace
        PS, _ps_free = tc.tile([3, 8], f32, space=MemorySpace.PSUM, name="PS")
        nc.tensor.matmul(PS[:], lhsT=W[:], rhs=ACC[:], start=True, stop=True)
        FIN = pool.tile([3, 8], f32, name="FIN")
        nc.vector.tensor_copy(FIN[:], PS[:])

        nc.sync.dma_start(out=out.ap(), in_=FIN[:])



def build_bass():
    global _CACHED_NC
    if _CACHED_NC is not None:
        return _CACHED_NC
    import concourse.bacc as bacc
    import concourse.tile as tile
    import concourse.mybir as mybir
    import concourse.bass_utils as _bu

    # The NEFF epilogue zeroes the whole sem file one EVENT_SEMAPHORE at a
    # time (~6us, serialized on the slowest engine). --max-sem-num=150 trims
    # the first few walrus-internal sems out of that clear set (3..6).
    if not hasattr(_bu, "_orig_get_walrus_args"):
        _bu._orig_get_walrus_args = _bu.get_walrus_args

        def _patched_walrus_args(*a, **k):
            return [*_bu._orig_get_walrus_args(*a, **k), "--max-sem-num=150"]

        _bu.get_walrus_args = _patched_walrus_args

    f32 = mybir.dt.float32
    bf16 = mybir.dt.bfloat16
    fp8 = mybir.dt.float8e3
    Act = mybir.ActivationFunctionType

    class FastTileContext(tile.TileContext):
        # TileContext._drain_and_barrier minus the end-of-kernel barrier and
        # the semaphore range-clear — the NEFF's own epilogue already runs a
        # pre-storm all-engine barrier, and with every kernel sem steered
        # into 207..255 the storm's Sync-engine portion (which runs after
        # Sync's drain, i.e. after the output DMA lands) covers the reset.
        def _drain_and_barrier(self, tick_clock, wait_clock):
            clock = tile.ScopedClock({None: tick_clock.global_clock})
            drain_inst = self.nc.sync.drain()
            wait_clock.add_sem_waits(drain_inst.ins, clock)
            popped = self.nc._tile_sem_poison_stack.pop()
            assert popped is self._sem_poison
            # No barrier and no explicit sem clear: the NEFF's own epilogue
            # runs a pre-storm all-engine barrier, and its Sync-engine clear
            # range (207..255) covers every sem this kernel uses.

    nc = bacc.Bacc("TRN2", target_bir_lowering=False, debug=False,
                   num_devices=NCORES)
    # Steer every tile/DMA semaphore into 207..255: keeps the live sems out
    # of the clear ranges the NEFF epilogue hands to the non-Sync engines.
    nc._state.reset_free_semaphores(list(range(207, 256)))
    obj = nc.dram_tensor("obj", [128, F_OBJ], fp8, kind="ExternalInput")
    cls_t = nc.dram_tensor("cls", [P_PAIRS, W_CLS], fp8, kind="ExternalInput")
    small = nc.dram_tensor("small", [P_PAIRS, 72], f32, kind="ExternalInput")
    out = nc.dram_tensor("partials", [3, 8], f32, kind="ExternalOutput")
    with FastTileContext(nc) as tc:
        _emit(nc, tc, mybir, obj, cls_t, small, out)

    # Route every Exp/Ln to the one table that holds both, so the kernel pays
    # a single ACT_TABLE_LOAD instead of ping-ponging between per-func tables.
    orig_tables = bacc.get_activation_tables

    def _merged_tables(arch):
        out_d = {}
        for name, s in orig_tables(arch).items():
            s2 = set(s)
            if name != "natural_log_exp_and_others":
                s2.discard(Act.Exp)
                s2.discard(Act.Ln)
            out_d[name] = s2
        return out_d

    bacc.get_activation_tables = _merged_tables
    try:
        nc.compile()
    finally:
        bacc.get_activation_tables = orig_tables

    # Drop the two dead const memsets (bf16 1.0, uint8 127 — this kernel
    # never reads them): the gpsimd const chain gates the tile-enter dance,
    # so each dead memset costs ~0.1us of every core's prologue.
    entry = nc.main_func.blocks[0]
    dead_consts = ("const-bfloat16-1.0", "const-uint8-127",
                   "const-float32-0.0", "const-float32-1.0")
    entry.instructions[:] = [
        ins for ins in entry.instructions
        if not (type(ins).__name__ == "InstMemset"
                and getattr(ins, "outs", None)
                and any(d in str(ins.outs[0]) for d in dead_consts))
    ]

    # Drop a spurious default-table InstLoadActFuncSet: when two loads appear
    # with no activation between them, the first is dead and its 1.3us sits
    # right before the first Exp on the critical path.
    for blk in nc.main_func.blocks:
        loads = []
        acts_seen = set()
        for idx, ins in enumerate(blk.instructions):
            tn = type(ins).__name__
            if tn == "InstLoadActFuncSet":
                loads.append((idx, ins))
            elif tn == "InstActivation":
                acts_seen.add(len(loads))
        if len(loads) == 2 and 1 not in acts_seen and loads[0][1].sync_info is None:
            blk.instructions.pop(loads[0][0])

    _CACHED_NC = nc
    return nc


def make_in_maps(pred_bbox, pred_obj, pred_cls, gt_boxes, gt_labels):
    import ml_dtypes

    fp8 = ml_dtypes.float8_e3m4
    labels = np.asarray(gt_labels).astype(np.int64)
    in_maps = []
    for core in range(NCORES):
        bs = slice(core * BPC, (core + 1) * BPC)

        po = np.asarray(pred_obj[bs], np.float32)
        obj = np.empty((128, F_OBJ), np.float32)
        obj[0:P_OBJ] = po.reshape(P_OBJ, F_OBJ)
        obj[P_OBJ] = -po[:, :M].reshape(F_OBJ)
        obj[P_OBJ + 1] = po[:, :M].reshape(F_OBJ)

        cl = np.asarray(pred_cls[bs, :M], np.float32).reshape(P_PAIRS, W_CLS)

        small = np.empty((P_PAIRS, 72), np.float32)
        pb = np.asarray(pred_bbox[bs, :M], np.float32).reshape(BPC, P_PAIRS, KP, 4)
        gb = np.asarray(gt_boxes[bs], np.float32).reshape(BPC, P_PAIRS, KP, 4)
        small[:, 0:32] = pb.transpose(1, 0, 2, 3).reshape(P_PAIRS, 32)
        small[:, 32:64] = gb.transpose(1, 0, 2, 3).reshape(P_PAIRS, 32)
        picked = np.take_along_axis(
            np.asarray(pred_cls[bs, :M], np.float32),
            labels[bs][..., None].astype(np.int64), axis=-1,
        )[..., 0]
        small[:, 64:72] = picked.reshape(P_PAIRS, NANCH)

        in_maps.append({
            "obj": obj.astype(fp8),
            "cls": cl.astype(fp8),
            "small": small,
        })
    return in_maps


def finalize(per_core_partials):
    s_iou = s_ratio = s_all = s_pos = s_posplus = s_lse = s_picked = 0.0
    for p in per_core_partials:
        p = p.astype(np.float64)
        s_iou += p[0, 0]
        s_ratio += p[0, 1]
        s_all += p[0, 2]
        s_pos += p[1, 2]
        s_posplus += p[2, 2]
        s_lse += p[0, 3]
        s_picked += p[0, 4]
    n_pos = B * M
    n_neg = B * (N - M)
    loss_bbox = 5.0 * (n_pos - s_iou + s_ratio) / n_pos
    loss_obj = s_pos / n_pos + 0.5 * (s_all - s_posplus) / n_neg
    loss_cls = (s_lse - s_picked) / n_pos
    total = loss_bbox + loss_obj + loss_cls
    return np.array([total, loss_bbox, loss_obj, loss_cls], dtype=np.float32)


def kernel(pred_bbox, pred_obj, pred_cls, gt_boxes, gt_labels):
    from concourse.bass_utils import run_bass_kernel_spmd

    nc = build_bass()
    in_maps = make_in_maps(pred_bbox, pred_obj, pred_cls, gt_boxes, gt_labels)
    res = run_bass_kernel_spmd(nc, in_maps, core_ids=list(range(NCORES)))
    return finalize([r["partials"] for r in res.results])
